# revision 2
# baseline (speedup 1.0000x reference)
"""DampedLinOSSLayer Trainium2 kernel v3 (8 NeuronCores, batch-sharded).

Gauge factorization: x_t = e^{i*th*t} y_t turns the complex diagonal
recurrence into a real-coefficient scan y_t = r y_{t-1} + c_t.
Structure per core (4 batches):
  - input shipped fp16; transposed to [h, t] by the DMA xbar (no PE work)
  - B-proj per 512-chunk on PE (fp16, chunk phase folded into weights)
  - pre-rotation: 2 broadcast muls + 1 add per (chunk, half) on DVE fp16
    (2x mode); tables hold the in-chunk phase
  - 4 full-length (2048) scans per batch on DVE; fp32 coefficient and
    internal state, fp16 in/out
  - post-rotation muls on DVE; the re/im recombination adds are folded
    into 8 accumulating C matmuls (+1 for the D residual) per chunk
  - output written [h, t] fp32; host does the final transpose
"""

import numpy as np

BATCH, LENGTH, HIDDEN, P = 32, 2048, 128, 256
N_CORES = 8
BPC = BATCH // N_CORES
CH = 512
NCH = LENGTH // CH

XIN_DTYPE = np.float16

_COMPILED = {}


def _build_program():
    import concourse.bacc as bacc
    import concourse.mybir as mybir
    from concourse.tile import TileContext

    f32 = mybir.dt.float32
    fp16 = mybir.dt.float16
    mm = mybir.AluOpType.mult
    ad = mybir.AluOpType.add

    nc = bacc.Bacc("TRN2", target_bir_lowering=False, debug=False,
                   num_devices=N_CORES)

    xin = nc.dram_tensor("xin", [BPC, LENGTH, HIDDEN], fp16,
                         kind="ExternalInput").ap()
    # B weights, phase-folded per chunk: [h, J, comp, half, p]
    bw = nc.dram_tensor("bw", [HIDDEN, NCH, 2, 2, 128], fp16,
                        kind="ExternalInput").ap()
    # C weights per chunk: [p, J, half, wt(0=C're,1=-C'im), h]
    cw = nc.dram_tensor("cw", [128, NCH, 2, 2, HIDDEN], fp16,
                        kind="ExternalInput").ap()
    # in-chunk rotation tables: [p, half, tab, comp, CH]
    epre = nc.dram_tensor("epre", [128, 2, 2, 2, CH], fp16,
                          kind="ExternalInput").ap()
    epost = nc.dram_tensor("epost", [128, 2, 2, 2, CH], fp16,
                           kind="ExternalInput").ap()
    rcol = nc.dram_tensor("rcol", [128, 2], f32, kind="ExternalInput").ap()
    dw = nc.dram_tensor("dw", [HIDDEN, HIDDEN], fp16,
                        kind="ExternalInput").ap()
    # output in [h, t] layout; host transposes
    out = nc.dram_tensor("out", [BPC, HIDDEN, LENGTH], f32,
                         kind="ExternalOutput").ap()

    with TileContext(nc) as tc:
        with (
            tc.tile_pool(name="const", bufs=1) as cpool,
            tc.tile_pool(name="intp", bufs=2) as intr_pool,
            tc.tile_pool(name="bub", bufs=2) as bub_pool,
            tc.tile_pool(name="t12", bufs=2) as t12_pool,
            tc.tile_pool(name="cbuf", bufs=2) as cbuf_pool,
            tc.tile_pool(name="ybuf", bufs=2) as ybuf_pool,
            tc.tile_pool(name="xbuf", bufs=2) as xbuf_pool,
            tc.tile_pool(name="obuf", bufs=2) as obuf_pool,
            tc.tile_pool(name="psb", bufs=3, space="PSUM") as psb,
            tc.tile_pool(name="pso", bufs=2, space="PSUM") as pso,
        ):
            bw_t = cpool.tile([HIDDEN, NCH, 2, 2, 128], fp16, tag="bw")
            cw_t = cpool.tile([128, NCH, 2, 2, HIDDEN], fp16, tag="cw")
            epre_t = cpool.tile([128, 2, 2, 2, CH], fp16, tag="epre")
            epost_t = cpool.tile([128, 2, 2, 2, CH], fp16, tag="epost")
            rcol_t = cpool.tile([128, 2], f32, tag="rcol")
            dw_t = cpool.tile([HIDDEN, HIDDEN], fp16, tag="dw")
            for src, dst in [(bw, bw_t), (cw, cw_t), (epre, epre_t),
                             (epost, epost_t), (rcol, rcol_t), (dw, dw_t)]:
                nc.sync.dma_start(dst[:], src[:])

            # scan coefficient, f32, broadcast along t: [128, half, LENGTH]
            rbc = cpool.tile([128, 2, LENGTH], f32, tag="rbc")
            for half in range(2):
                nc.vector.memset(rbc[:, half], 1.0)
                nc.vector.tensor_scalar_mul(
                    rbc[:, half], rbc[:, half], rcol_t[:, half:half + 1])

            for b in range(BPC):
                # ---- input transpose via DMA xbar: inT [h, t] fp16 ----
                inT = intr_pool.tile([HIDDEN, LENGTH], fp16, tag="inT")
                nc.sync.dma_start_transpose(inT[:], xin[b])

                # ---- B-proj + pre-rotation into cf streams ----
                cf = [cbuf_pool.tile([128, 2, LENGTH], fp16, tag=f"cf{h}",
                                     name=f"cf{h}")
                      for h in range(2)]
                for J in range(NCH):
                    tsl = slice(CH * J, CH * (J + 1))
                    for half in range(2):
                        bu = psb.tile([128, 2, CH], f32, tag="bu")
                        for comp in range(2):
                            nc.tensor.matmul(
                                bu[:, comp, :], bw_t[:, J, comp, half],
                                inT[:, tsl], start=True, stop=True)
                        bub = bub_pool.tile([128, 2, CH], fp16, tag="bub")
                        nc.scalar.copy(bub[:], bu[:])
                        # A = bur (bcast) * [cos | -sin] ; B = bui * [sin | cos]
                        # cf[cre|cim] = A + B, all operands contiguous
                        t12 = t12_pool.tile([128, 2, 2, CH], fp16, tag="t12")
                        nc.vector.tensor_mul(
                            t12[:, 0],
                            bub[:, 0:1, :].to_broadcast([128, 2, CH]),
                            epre_t[:, half, 0])
                        nc.vector.tensor_mul(
                            t12[:, 1],
                            bub[:, 1:2, :].to_broadcast([128, 2, CH]),
                            epre_t[:, half, 1])
                        nc.vector.tensor_add(cf[half][:, :, tsl], t12[:, 0],
                                             t12[:, 1])

                # ---- scans: y[half][comp, :] over full length ----
                yt = [ybuf_pool.tile([128, 2, LENGTH], fp16, tag=f"y{h}",
                                     name=f"y{h}")
                      for h in range(2)]
                for half in range(2):
                    for comp in range(2):
                        nc.vector.tensor_tensor_scan(
                            yt[half][:, comp, :], rbc[:, half],
                            cf[half][:, comp, :], 0.0, op0=mm, op1=ad)

                # ---- post-rotation + C-proj + D ----
                for J in range(NCH):
                    tsl = slice(CH * J, CH * (J + 1))
                    outT = pso.tile([HIDDEN, CH], f32, tag="outT")
                    first = True
                    for half in range(2):
                        # t3 = y*[cos|-sin] -> weights C're ; t4 = y*[sin|cos] -> -C'im
                        t3 = xbuf_pool.tile([128, 2, CH], fp16, tag="t3")
                        t4 = xbuf_pool.tile([128, 2, CH], fp16, tag="t4")
                        nc.vector.tensor_mul(t3[:], yt[half][:, :, tsl],
                                             epost_t[:, half, 0])
                        nc.vector.tensor_mul(t4[:], yt[half][:, :, tsl],
                                             epost_t[:, half, 1])
                        for wt, tt in ((0, t3), (1, t4)):
                            for comp in range(2):
                                nc.tensor.matmul(
                                    outT[:], cw_t[:, J, half, wt],
                                    tt[:, comp, :],
                                    start=first, stop=False)
                                first = False
                    nc.tensor.matmul(outT[:], dw_t[:], inT[:, tsl],
                                     start=False, stop=True)
                    oT = obuf_pool.tile([HIDDEN, CH], f32, tag="oT")
                    nc.scalar.copy(oT[:], outT[:])
                    nc.sync.dma_start(out[b, :, tsl], oT[:])

    nc.compile()
    return nc


def _host_constants(A_diag, G_diag, steps, B, C, D):
    A = A_diag.astype(np.float64)
    G = G_diag.astype(np.float64)
    st = steps.astype(np.float64)
    step = 1.0 / (1.0 + np.exp(-st))
    g = np.maximum(G, 0.0)
    denom = np.maximum(step * step, 1e-6)
    s = step * g
    base = np.sqrt(np.maximum(1.0 + s, 1e-6))
    a_low = (2.0 + s - 2.0 * base) / denom
    a_high = (2.0 + s + 2.0 * base) / denom
    a = a_low + np.maximum(A - a_low, 0.0) - np.maximum(A - a_high, 0.0)
    S = 1.0 / (1.0 + step * g)
    T = S + 1.0 - step * step * S * a
    imag = np.sqrt(np.maximum(S - 0.25 * T * T, 0.0))
    lam = 0.5 * T + 1j * imag
    r = np.abs(lam)
    th = np.angle(lam)

    j0 = np.arange(CH, dtype=np.float64)
    cos_m = np.cos(th[:, None] * j0[None, :])
    sin_m = np.sin(th[:, None] * j0[None, :])

    # epre: tab0 applied to broadcast(bur): [cos | -sin]
    #       tab1 applied to broadcast(bui): [sin | cos]
    #   so cf = A + B gives cre = bur*cos + bui*sin ;
    #                       cim = -bur*sin + bui*cos
    # epost planes for the C-matmul fold:
    #   tab0 (weights C're): [cos | -sin] ; tab1 (weights -C'im): [sin | cos]
    epre = np.zeros((128, 2, 2, 2, CH), np.float16)
    epost = np.zeros((128, 2, 2, 2, CH), np.float16)
    for half in range(2):
        psl = slice(128 * half, 128 * (half + 1))
        epre[:, half, 0, 0] = cos_m[psl]
        epre[:, half, 0, 1] = -sin_m[psl]
        epre[:, half, 1, 0] = sin_m[psl]
        epre[:, half, 1, 1] = cos_m[psl]
        epost[:, half, 0, 0] = cos_m[psl]
        epost[:, half, 0, 1] = -sin_m[psl]
        epost[:, half, 1, 0] = sin_m[psl]
        epost[:, half, 1, 1] = cos_m[psl]

    Bc = B[..., 0].astype(np.float64) + 1j * B[..., 1].astype(np.float64)
    Cc = C[..., 0].astype(np.float64) + 1j * C[..., 1].astype(np.float64)
    bw = np.zeros((HIDDEN, NCH, 2, 2, 128), np.float16)
    cwt = np.zeros((128, NCH, 2, 2, HIDDEN), np.float16)
    for J in range(NCH):
        ph = np.exp(-1j * th * (CH * J))
        BJ = Bc * ph[:, None]
        phc = np.exp(+1j * th * (CH * J))
        CT = Cc * phc[None, :]                     # [H, P]
        for half in range(2):
            psl = slice(128 * half, 128 * (half + 1))
            bw[:, J, 0, half] = BJ.real[psl].T
            bw[:, J, 1, half] = BJ.imag[psl].T
            # wt 0: C're ; wt 1: -C'im   (lhsT [p, h])
            cwt[:, J, half, 0] = CT.real[:, psl].T
            cwt[:, J, half, 1] = -CT.imag[:, psl].T

    rcol = np.zeros((128, 2), np.float32)
    rcol[:, 0] = r[:128]
    rcol[:, 1] = r[128:]
    dwm = np.diag(D.astype(np.float64)).astype(np.float16)
    return dict(bw=bw, cw=cwt, epre=epre, epost=epost, rcol=rcol, dw=dwm)


def kernel(inputs, A_diag, G_diag, steps, B, C, D):
    from concourse import bass_utils

    inputs = np.asarray(inputs, np.float32)
    consts = _host_constants(np.asarray(A_diag), np.asarray(G_diag),
                             np.asarray(steps), np.asarray(B), np.asarray(C),
                             np.asarray(D))

    if "prog" not in _COMPILED:
        _COMPILED["prog"] = _build_program()
    nc = _COMPILED["prog"]

    in_maps = []
    for core in range(N_CORES):
        m = dict(consts)
        m["xin"] = np.ascontiguousarray(
            inputs[BPC * core: BPC * (core + 1)]).astype(np.float16)
        in_maps.append(m)
    res = bass_utils.run_bass_kernel_spmd(nc, in_maps,
                                          core_ids=list(range(N_CORES)))
    out = np.concatenate([res.results[i]["out"] for i in range(N_CORES)],
                         axis=0)                      # [B, H, L]
    return np.ascontiguousarray(out.transpose(0, 2, 1)).astype(np.float32)
